# revision 1
# baseline (speedup 1.0000x reference)
"""DiffCLIP differential-attention block on 8 Trainium2 NeuronCores.

Sharding: the (batch=4) x (head-group=2) grid maps to the 8 cores — core
c = 2*b + g handles batch b and half the heads (8 of 16 q/k heads = 4 of 8
effective heads), i.e. a 512-column slice of the q/k/v projections and the
matching 512-row slice of the out projection. Each core emits a partial
(L, D) output; the host sums the two per-batch partials and stacks.

Per-core pipeline (all matmuls bf16 with fp32 PSUM accumulation):
  A: qT = (Wq*scale)^T x, kT = Wk^T x (transposed: channels on partitions),
     v = x Wv (natural: tokens on partitions)
  B: per head pair, transposed scores sT = k q^T per side, exp on ScalarE
  C: row sums via ones-matmul (replicated across partitions), reciprocal,
     u = v^T e per side, z = u0*r0 - lam*u1*r1, RMS via ones-matmul of z^2
  D: y = normed^T-weighted Wo_eff (norm_w and (1-lambda_init) folded in)
"""

import sys

if "/opt/trn_rl_repo" not in sys.path:
    sys.path.insert(0, "/opt/trn_rl_repo")

import numpy as np
import ml_dtypes

L, D, H, HD, HE = 1024, 1024, 16, 64, 8
LAMBDA_INIT = 0.8
EPS = 1e-5
NB = 4
NCORES = 8
COLS = 512  # per-core projection column count

LAST_RESULT = None  # BassKernelResults of the most recent kernel() call


def _split_excess_waits(nc, max_waits: int = 1):
    """Walrus codegen on this toolchain accepts at most one sync-wait command
    per hardware instruction (plus its update); Tile freely emits several.
    Split the excess waits onto preceding same-engine NoOps."""
    import bass_rust
    import concourse.mybir as mybir

    for f in nc.m.functions:
        for blk in f.blocks:
            insts = blk.instructions
            out = []
            changed = False
            for inst in insts:
                si = inst.sync_info
                if si is not None and si.on_wait and len(si.on_wait) > max_waits:
                    waits = list(si.on_wait)
                    for j, w in enumerate(waits[max_waits:]):
                        nop = mybir.InstNoOp(
                            name=f"{inst.name}-xw{j}",
                            sync_info=bass_rust.SyncInfo(
                                on_wait=[w], on_update=[]
                            ),
                            bass_nofuse=True,
                            engine=inst.engine,
                        )
                        nc.register_instruction(nop, overwrite=True)
                        out.append(nop)
                    inst.sync_info = bass_rust.SyncInfo(
                        on_wait=waits[:max_waits],
                        on_update=list(si.on_update or []),
                    )
                    changed = True
                out.append(inst)
            if changed:
                blk.instructions = out


def _build(lam: float, with_mask: bool, with_qk_bias: bool, with_v_bias: bool, split_waits: bool = True):
    import concourse.bass as bass
    import concourse.tile as tile
    import concourse.mybir as mybir

    bf16 = mybir.dt.bfloat16
    f32 = mybir.dt.float32
    AF = mybir.ActivationFunctionType
    ALU = mybir.AluOpType

    nc = bass.Bass()
    xT_d = nc.dram_tensor("xT", [D, L], bf16, kind="ExternalInput")
    wq_d = nc.dram_tensor("wq", [D, COLS], bf16, kind="ExternalInput")
    wk_d = nc.dram_tensor("wk", [D, COLS], bf16, kind="ExternalInput")
    wv_d = nc.dram_tensor("wv", [D, COLS], bf16, kind="ExternalInput")
    wo_d = nc.dram_tensor("wo", [COLS, D], bf16, kind="ExternalInput")
    if with_qk_bias:
        bq_d = nc.dram_tensor("bqs", [COLS], f32, kind="ExternalInput")
        bk_d = nc.dram_tensor("bks", [COLS], f32, kind="ExternalInput")
    if with_v_bias:
        bv_d = nc.dram_tensor("bvs", [COLS], f32, kind="ExternalInput")
    if with_mask:
        maskT_d = nc.dram_tensor("maskT", [L, L], bf16, kind="ExternalInput")
    y_d = nc.dram_tensor("y", [L, D], f32, kind="ExternalOutput")

    with tile.TileContext(nc) as tc:
        persist = tc.alloc_tile_pool(name="persist", bufs=1)
        qT = persist.tile([128, 4, L], bf16)
        kT = persist.tile([128, 4, L], bf16)
        v = persist.tile([128, 8, COLS], bf16)
        wo_s = persist.tile([128, 4, D], bf16)
        outT = persist.tile([128, 4, L], bf16)
        ones = persist.tile([128, 128], bf16)
        nc.vector.memset(ones, 1.0)
        onesc = persist.tile([1, 128], f32)
        nc.vector.memset(onesc, 1.0)
        nc.sync.dma_start(wo_s, wo_d.rearrange("(ko p) n -> p ko n", p=128))
        if with_mask:
            # identity used to add maskT into score PSUM via a matmul
            ident = persist.tile([128, 128], bf16)
            from concourse.masks import make_identity
            make_identity(nc, ident)
            maskT_s = persist.tile([128, 8, L], bf16)
            nc.sync.dma_start(
                maskT_s, maskT_d.rearrange("(ko p) l -> p ko l", p=128)
            )

        # ---- Stage A (projections) interleaved with attention pairs ----
        with (
            tc.tile_pool(name="stage_a", bufs=1) as sa,
            tc.tile_pool(name="epool", bufs=2) as ep,
            tc.tile_pool(name="tmp", bufs=2) as tp,
            tc.tile_pool(name="tmp1", bufs=1) as tp1,
            tc.tile_pool(name="drp", bufs=2, space="DRAM") as drp,
            tc.tile_pool(name="psA", bufs=2, space="PSUM") as psA,
            tc.tile_pool(name="psS", bufs=2, space="PSUM") as psS,
            tc.tile_pool(name="psR", bufs=1, space="PSUM") as psR,
            tc.tile_pool(name="psU", bufs=2, space="PSUM") as psU,
        ):
            xts = sa.tile([128, 8, L], bf16)
            wq_s = sa.tile([128, 8, COLS], bf16)
            wk_s = sa.tile([128, 8, COLS], bf16)
            wv_s = sa.tile([128, 8, COLS], bf16)
            # chunked loads so the first projection matmuls start early
            xT_r = xT_d.rearrange("(ko p) l -> p ko l", p=128)
            wq_r = wq_d.rearrange("(ko p) m -> p ko m", p=128)
            wk_r = wk_d.rearrange("(ko p) m -> p ko m", p=128)
            wv_r = wv_d.rearrange("(ko p) m -> p ko m", p=128)
            for kb in range(8):
                nc.sync.dma_start(wq_s[:, kb], wq_r[:, kb])
                nc.sync.dma_start(xts[:, kb], xT_r[:, kb])
                nc.sync.dma_start(wk_s[:, kb], wk_r[:, kb])
            for kb in range(8):
                nc.sync.dma_start(wv_s[:, kb], wv_r[:, kb])
            if with_qk_bias:
                bq_s = sa.tile([128, 4], f32)
                bk_s = sa.tile([128, 4], f32)
                nc.sync.dma_start(bq_s, bq_d.rearrange("(mb p) -> p mb", p=128))
                nc.sync.dma_start(bk_s, bk_d.rearrange("(mb p) -> p mb", p=128))
            if with_v_bias:
                bv_s = sa.tile([128, COLS], f32)
                bv_ap = bv_d[:]
                nc.gpsimd.dma_start(
                    bv_s,
                    bass.AP(
                        tensor=bv_ap.tensor,
                        offset=bv_ap.offset,
                        ap=[[0, 128], list(bv_ap.ap[0])],
                    ),
                )

            def emit_qkT(mb):
                # qT / kT columns [128*mb, 128*mb+128): channels on partitions
                for wt_s, dst, bias in ((wq_s, qT, "q"), (wk_s, kT, "k")):
                    for lc in range(2):
                        acc = psA.tile([128, 512], f32, tag="accA")
                        for kb in range(8):
                            nc.tensor.matmul(
                                acc[:],
                                wt_s[:, kb, mb * 128:(mb + 1) * 128],
                                xts[:, kb, lc * 512:(lc + 1) * 512],
                                start=(kb == 0),
                                stop=(kb == 7),
                            )
                        dst_ap = dst[:, mb, lc * 512:(lc + 1) * 512]
                        if with_qk_bias:
                            b_s = bq_s if bias == "q" else bk_s
                            nc.scalar.activation(
                                out=dst_ap, in_=acc[:], func=AF.Identity,
                                bias=b_s[:, mb:mb + 1], scale=1.0,
                            )
                        else:
                            nc.vector.tensor_copy(dst_ap, acc[:])

            def emit_v():
                # v: tokens on partitions
                for lb in range(8):
                    acc = psA.tile([128, 512], f32, tag="accA")
                    for kb in range(8):
                        nc.tensor.matmul(
                            acc[:],
                            xts[:, kb, lb * 128:(lb + 1) * 128],
                            wv_s[:, kb, :],
                            start=(kb == 0),
                            stop=(kb == 7),
                        )
                    if with_v_bias:
                        nc.vector.tensor_add(v[:, lb, :], acc[:], bv_s[:])
                    else:
                        nc.vector.tensor_copy(v[:, lb, :], acc[:])

            def emit_scores(g):
                e0 = ep.tile([128, 8, L], bf16, tag="e0")
                e1 = ep.tile([128, 8, L], bf16, tag="e1")
                es = (e0, e1)
                for kb in range(8):
                    for lc in range(2):
                        for s in range(2):
                            sc = psS.tile([128, 512], f32, tag="sc")
                            nc.tensor.matmul(
                                sc[:],
                                kT[64 * s:64 * (s + 1), g, kb * 128:(kb + 1) * 128],
                                qT[64 * s:64 * (s + 1), g, lc * 512:(lc + 1) * 512],
                                start=True,
                                stop=not with_mask,
                            )
                            if with_mask:
                                nc.tensor.matmul(
                                    sc[:],
                                    ident[:],
                                    maskT_s[:, kb, lc * 512:(lc + 1) * 512],
                                    start=False,
                                    stop=True,
                                )
                            nc.scalar.activation(
                                out=es[s][:, kb, lc * 512:(lc + 1) * 512],
                                in_=sc[:],
                                func=AF.Exp,
                            )
                return es

            def emit_attend(g, es):
                # Division-free combine: z' = u0*s1 - lam*u1*s0, then
                # normed = z' * rsqrt(mean(z'^2) + eps*(s0*s1)^2).
                s1_sb = tp.tile([128, L], f32, tag="s1sb")
                s0l_sb = tp.tile([128, L], f32, tag="s0lsb")  # lam * s0
                for s in range(2):
                    # per-k'-block partial sums: bf16 tree adds on DVE (2x
                    # packed), then a small ones-matmul reduces the partition
                    # dim, landing replicated across partitions.
                    e = es[s]
                    t4 = tp1.tile([128, 4, L], bf16, tag="tsum4")
                    for j in range(4):
                        nc.vector.tensor_add(
                            t4[:, j], e[:, 2 * j], e[:, 2 * j + 1]
                        )
                    t2 = tp1.tile([128, 2, L], bf16, tag="tsum2")
                    nc.vector.tensor_add(t2[:, 0], t4[:, 0], t4[:, 1])
                    nc.vector.tensor_add(t2[:, 1], t4[:, 2], t4[:, 3])
                    pe_s = tp1.tile([128, L], bf16, tag="pesum")
                    nc.vector.tensor_add(pe_s[:], t2[:, 0], t2[:, 1])
                    srep = psR.tile([128, L], f32, tag="srep")
                    for lc in range(2):
                        nc.tensor.matmul(
                            srep[:, lc * 512:(lc + 1) * 512],
                            ones[:],
                            pe_s[:, lc * 512:(lc + 1) * 512],
                            start=True,
                            stop=True,
                        )
                    if s == 0:
                        nc.vector.tensor_scalar_mul(s0l_sb[:], srep[:], lam)
                    else:
                        nc.vector.tensor_copy(s1_sb[:], srep[:])
                # rows of the replicated sums -> DRAM -> [128, 8] smalls
                scr0 = drp.tile([L], f32, tag="r0")
                nc.sync.dma_start(scr0[:], s0l_sb[0:1, :])
                scr1 = drp.tile([L], f32, tag="r1")
                nc.sync.dma_start(scr1[:], s1_sb[0:1, :])
                s0l_sm = tp.tile([128, 8], f32, tag="s0lsm")
                nc.sync.dma_start(s0l_sm[:], scr0.rearrange("(p f) -> p f", p=128))
                s1_sm = tp.tile([128, 8], f32, tag="s1sm")
                nc.sync.dma_start(s1_sm[:], scr1.rearrange("(p f) -> p f", p=128))
                c_sm = tp.tile([128, 8], f32, tag="csm")
                nc.vector.tensor_mul(c_sm[:], s0l_sm[:], s1_sm[:])
                csq_sm = tp.tile([128, 8], f32, tag="csqsm")
                nc.vector.tensor_mul(csq_sm[:], c_sm[:], c_sm[:])

                z = tp.tile([128, L], f32, tag="z")
                for lc in range(2):
                    us = []
                    for s in range(2):
                        u = psU.tile([128, 512], f32, tag="u")
                        for kb in range(8):
                            nc.tensor.matmul(
                                u[:],
                                v[:, kb, 128 * g:128 * (g + 1)],
                                es[s][:, kb, lc * 512:(lc + 1) * 512],
                                start=(kb == 0),
                                stop=(kb == 7),
                            )
                        us.append(u)
                    t0 = tp1.tile([128, 512], f32, tag="t0")
                    nc.vector.tensor_mul(
                        t0[:], us[0][:], s1_sb[:, lc * 512:(lc + 1) * 512]
                    )
                    t1 = tp1.tile([128, 512], f32, tag="t1")
                    nc.vector.tensor_mul(
                        t1[:], us[1][:], s0l_sb[:, lc * 512:(lc + 1) * 512]
                    )
                    nc.vector.tensor_sub(
                        z[:, lc * 512:(lc + 1) * 512], t0[:], t1[:]
                    )
                # headwise RMS over the 128 partition dim via ones-matmul of z^2
                zsq = tp.tile([128, L], bf16, tag="zsq")
                nc.vector.tensor_mul(zsq[:], z[:], z[:])
                sq = psR.tile([128, L], f32, tag="srep")
                for lc in range(2):
                    nc.tensor.matmul(
                        sq[:, lc * 512:(lc + 1) * 512],
                        ones[:],
                        zsq[:, lc * 512:(lc + 1) * 512],
                        start=True,
                        stop=True,
                    )
                sq_row = tp1.tile([1, L], f32, tag="sqrow")
                nc.vector.tensor_copy(sq_row[:], sq[0:1, :])
                scrq = drp.tile([L], f32, tag="rq")
                nc.sync.dma_start(scrq[:], sq_row[0:1, :])
                sq_sm = tp.tile([128, 8], f32, tag="sqsm")
                nc.sync.dma_start(sq_sm[:], scrq.rearrange("(p f) -> p f", p=128))
                # smalls: w*128 = sq + (128*eps/lam^2) * (lam*s0*s1)^2
                w_sm = tp.tile([128, 8], f32, tag="wsm")
                nc.vector.scalar_tensor_tensor(
                    out=w_sm[:],
                    in0=csq_sm[:],
                    scalar=128.0 * EPS / (lam * lam),
                    in1=sq_sm[:],
                    op0=ALU.mult,
                    op1=ALU.add,
                )
                srt_sm = tp.tile([128, 8], f32, tag="srtsm")
                nc.scalar.activation(
                    out=srt_sm[:], in_=w_sm[:], func=AF.Sqrt, scale=1.0 / 128,
                )
                rsq_sm = tp.tile([128, 8], f32, tag="rsqsm")
                nc.vector.reciprocal(rsq_sm[:], srt_sm[:])
                scrr = drp.tile([L], f32, tag="rr")
                nc.sync.dma_start(scrr.rearrange("(p f) -> p f", p=128), rsq_sm[:])
                rsq_row = tp1.tile([1, L], f32, tag="rsqrow")
                nc.sync.dma_start(rsq_row[:], scrr[:])
                # broadcast across partitions via a K=1 fp32 ones-matmul,
                # fused into the final normed multiply per 512-chunk
                for lc in range(2):
                    rep = psS.tile([128, 512], f32, tag="sc")
                    nc.tensor.matmul(
                        rep[:],
                        onesc[:],
                        rsq_row[0:1, lc * 512:(lc + 1) * 512],
                        start=True,
                        stop=True,
                    )
                    nc.vector.tensor_mul(
                        outT[:, g, lc * 512:(lc + 1) * 512],
                        z[:, lc * 512:(lc + 1) * 512],
                        rep[:],
                    )

            # interleaved emission: hide stage A behind pair 0/1 activity
            emit_qkT(0)
            es0 = emit_scores(0)
            emit_qkT(1)
            emit_v()
            es1 = emit_scores(1)
            emit_qkT(2)
            emit_attend(0, es0)
            es2 = emit_scores(2)
            emit_qkT(3)
            emit_attend(1, es1)
            es3 = emit_scores(3)
            emit_attend(2, es2)
            emit_attend(3, es3)

        # ---------------- Stage D: output projection ----------------
        with (
            tc.tile_pool(name="yp", bufs=3) as yp,
            tc.tile_pool(name="psY", bufs=2, space="PSUM") as psY,
        ):
            y_r = y_d.rearrange("(lb p) n -> p lb n", p=128)
            for lb in range(8):
                for nk in range(2):
                    acc = psY.tile([128, 512], f32, tag="y")
                    for g in range(4):
                        nc.tensor.matmul(
                            acc[:],
                            outT[:, g, lb * 128:(lb + 1) * 128],
                            wo_s[:, g, nk * 512:(nk + 1) * 512],
                            start=(g == 0),
                            stop=(g == 3),
                        )
                    yt = yp.tile([128, 512], f32, tag="yt")
                    nc.scalar.copy(out=yt[:], in_=acc[:])
                    nc.sync.dma_start(
                        y_r[:, lb, nk * 512:(nk + 1) * 512], yt[:]
                    )

        persist.release()
    if split_waits:
        _split_excess_waits(nc)
    return nc


def kernel(**inputs) -> np.ndarray:
    from concourse.bass_utils import run_bass_kernel_spmd

    bf = ml_dtypes.bfloat16
    q_in = np.asarray(inputs["query"], np.float32)      # (L, NB, D)
    Wq = np.asarray(inputs["Wq"], np.float32)
    Wk = np.asarray(inputs["Wk"], np.float32)
    Wv = np.asarray(inputs["Wv"], np.float32)
    Wo = np.asarray(inputs["Wo"], np.float32)
    bq = np.asarray(inputs["bq"], np.float32)
    bk = np.asarray(inputs["bk"], np.float32)
    bv = np.asarray(inputs["bv"], np.float32)
    bo = np.asarray(inputs["bo"], np.float32)
    norm_w = np.asarray(inputs["norm_w"], np.float32)
    mask = np.asarray(inputs["attn_mask"], np.float32)
    lq1 = np.asarray(inputs["lq1"], np.float32)
    lk1 = np.asarray(inputs["lk1"], np.float32)
    lq2 = np.asarray(inputs["lq2"], np.float32)
    lk2 = np.asarray(inputs["lk2"], np.float32)

    lam = float(
        np.exp(np.sum(lq1 * lk1)) - np.exp(np.sum(lq2 * lk2)) + LAMBDA_INIT
    )
    scale = HD ** -0.5
    with_mask = bool(np.any(mask))
    with_qk_bias = bool(np.any(bq) or np.any(bk))
    with_v_bias = bool(np.any(bv))
    # norm_w * (1 - lambda_init) folded into Wo rows (tiled per he-head)
    nw = np.tile(norm_w * (1.0 - LAMBDA_INIT), HE // 2)  # (COLS,)

    nc = _build(lam, with_mask, with_qk_bias, with_v_bias)

    maskT = np.ascontiguousarray(mask.T).astype(bf) if with_mask else None
    in_maps = []
    for c in range(NCORES):
        b, g2 = divmod(c, 2)
        cols = slice(COLS * g2, COLS * (g2 + 1))
        x = q_in[:, b, :]
        im = {
            "xT": np.ascontiguousarray(x.T).astype(bf),
            "wq": (Wq[:, cols] * scale).astype(bf),
            "wk": np.ascontiguousarray(Wk[:, cols]).astype(bf),
            "wv": np.ascontiguousarray(Wv[:, cols]).astype(bf),
            "wo": (Wo[cols, :] * nw[:, None]).astype(bf),
        }
        if with_qk_bias:
            im["bqs"] = np.ascontiguousarray(bq[cols] * scale)
            im["bks"] = np.ascontiguousarray(bk[cols])
        if with_v_bias:
            im["bvs"] = np.ascontiguousarray(bv[cols])
        if with_mask:
            im["maskT"] = maskT
        in_maps.append(im)

    res = run_bass_kernel_spmd(nc, in_maps, core_ids=list(range(NCORES)))
    global LAST_RESULT
    LAST_RESULT = res
    outs = [r["y"] for r in res.results]

    out = np.empty((L, NB, D), np.float32)
    for b in range(NB):
        yb = outs[2 * b] + outs[2 * b + 1]
        if np.any(bo):
            yb = yb + bo
        out[:, b, :] = yb
    return out

